# revision 1
# baseline (speedup 1.0000x reference)
"""Trainium2 Bass kernel for a 4-layer causal decoder (nn_CachedDecoder).

Model: B=4, S=2048, D=256, H=8 heads (d_k=32), 4 attention layers (no
residual), final LayerNorm, vocab head (V=1000).

Sharding (8 cores, SPMD single program):
  core c -> (batch b = c//2, head-group g = c%2 of 4 heads = 128 att dims).
  Each core runs all 2048 positions for its 4 heads; the Wo projection
  produces a partial h which is AllReduce-summed within the core pair
  (replica groups [2b, 2b+1]) between layers, one collective per 512-wide
  q-tile so collectives overlap with attention compute.  After the last layer a
  ReduceScatter (split into two quarter-collectives) hands each core half
  the positions for LN + vocab head, chained per 512-chunk for overlap.

Layouts (SBUF):
  hT   [128 i_in, 2 i_out, S]   (transposed residual stream, fp32r)
  QT/KT[128 (head,dk), S]       fp32r; head h on partitions 32h..32h+32
  V    [128 k_in, 16 k_blk, 128 (head,dk)]  fp16
  P    [128 k, 2 head, 512 q]   fp16 (softmax numerators; 2 heads per PSUM
       scores tile so the 8-bank budget allows double buffering — the PE
       still packs all 4 heads via disjoint tile_position row/col groups)
Scores are computed transposed ([k, q]) so softmax sums reduce over the
partition axis via an all-ones matmul (which also replicates each head's
denominator across its 32 rows for the later divide).  Causal masking adds a
-1e4 staircase to the scores in PSUM via an identity matmul; masked
probabilities underflow to exactly 0 in the exp.
"""

import math

import numpy as np

D_MODEL = 256
NUM_HEADS = 8
NUM_LAYERS = 4
VOCAB = 1000
D_K = D_MODEL // NUM_HEADS  # 32
LN_EPS = 1e-5
B = 4
S = 2048
N_CORES = 8
HPG = 4            # heads per group (per core)
GD = HPG * D_K     # 128 att dims per core
SCALE = 1.0 / math.sqrt(D_K)
QT_W = 512         # q-tile width
NQT = S // QT_W    # 4
KB = 128           # k-block
NKB = S // KB      # 16
S_HALF = S // 2
MASK_NEG = -1e4

_prog_cache = {}


def _build_program():
    import concourse.bacc as bacc
    import concourse.mybir as mybir
    import concourse.tile as tile

    F32 = mybir.dt.float32
    F32R = mybir.dt.float32r
    BF16 = mybir.dt.float16
    Exp = mybir.ActivationFunctionType.Exp
    Sqrt = mybir.ActivationFunctionType.Sqrt
    MUL = mybir.AluOpType.mult
    ADD = mybir.AluOpType.add
    SUB = mybir.AluOpType.subtract

    nc = bacc.Bacc("TRN2", target_bir_lowering=False, debug=False,
                   num_devices=N_CORES)

    h0t = nc.dram_tensor("h0t", [128, 2, S], F32R, kind="ExternalInput")
    wq = nc.dram_tensor("wq", [NUM_LAYERS, 128, 2, GD], F32R, kind="ExternalInput")
    wk = nc.dram_tensor("wk", [NUM_LAYERS, 128, 2, GD], F32R, kind="ExternalInput")
    wv = nc.dram_tensor("wv", [NUM_LAYERS, 128, 2, GD], F32R, kind="ExternalInput")
    wo = nc.dram_tensor("wo", [NUM_LAYERS, 128, 2, 128], F32R, kind="ExternalInput")
    whd = nc.dram_tensor("whd", [128, 2, VOCAB], F32R, kind="ExternalInput")
    wsum = nc.dram_tensor("wsum", [1, VOCAB], F32R, kind="ExternalInput")
    bconst = nc.dram_tensor("bconst", [1, VOCAB], F32R, kind="ExternalInput")
    masks = nc.dram_tensor("masks", [128, 128], BF16, kind="ExternalInput")
    identb = nc.dram_tensor("identb", [128, 128], BF16, kind="ExternalInput")
    ones32 = nc.dram_tensor("ones32", [128, 32], BF16, kind="ExternalInput")
    ones1r = nc.dram_tensor("ones1r", [128, 1], F32R, kind="ExternalInput")
    y = nc.dram_tensor("y", [S_HALF, VOCAB], F32, kind="ExternalOutput")

    groups = [[2 * b, 2 * b + 1] for b in range(B)]

    with tile.TileContext(nc) as tc:
        with (
            tc.tile_pool(name="consts", bufs=1) as consts,
            tc.tile_pool(name="hpool", bufs=2) as hpool,
            tc.tile_pool(name="qkv", bufs=2) as qkv,
            tc.tile_pool(name="ppool", bufs=8) as ppool,
            tc.tile_pool(name="att", bufs=4) as attp,
            tc.tile_pool(name="small", bufs=1) as small,
            tc.tile_pool(name="logitp", bufs=2) as logitp,
            tc.tile_pool(name="ps_pr", bufs=2, space="PSUM") as ps_pr,
            tc.tile_pool(name="dram", bufs=2, space="DRAM") as dram,
        ):
            # ---- constants / weights resident in SBUF ----
            # emission order is the scheduler's priority hint: h0 and layer-0
            # weights first so layer 0 isn't gated on the full weight load
            h_sb = hpool.tile([128, 2, S], F32R, tag="h")
            for st in range(NQT):
                sl = slice(st * QT_W, (st + 1) * QT_W)
                nc.sync.dma_start(h_sb[:, :, sl], h0t[:, :, sl])
            wq_sb = consts.tile([128, NUM_LAYERS, 2, GD], F32R)
            wk_sb = consts.tile([128, NUM_LAYERS, 2, GD], F32R)
            wv_sb = consts.tile([128, NUM_LAYERS, 2, GD], F32R)
            wo_sb = consts.tile([128, NUM_LAYERS, 2, 128], F32R)
            for l in range(NUM_LAYERS):
                nc.sync.dma_start(wq_sb[:, l, :, :], wq[l, :, :, :])
                nc.sync.dma_start(wk_sb[:, l, :, :], wk[l, :, :, :])
                nc.sync.dma_start(wv_sb[:, l, :, :], wv[l, :, :, :])
                nc.sync.dma_start(wo_sb[:, l, :, :], wo[l, :, :, :])
                if l == 0:
                    ident_sb = consts.tile([128, 128], BF16)
                    nc.sync.dma_start(ident_sb[:], identb[:])
                    ones32_sb = consts.tile([128, 32], BF16)
                    nc.sync.dma_start(ones32_sb[:], ones32[:])
                    ones1_sb = consts.tile([128, 1], F32R)
                    nc.sync.dma_start(ones1_sb[:], ones1r[:])
                    masks_sb = consts.tile([128, 128], BF16)
                    nc.sync.dma_start(masks_sb[:], masks[:])
            whd_sb = consts.tile([128, 2, VOCAB], F32R)
            nc.sync.dma_start(whd_sb[:], whd[:])
            wsum_sb = consts.tile([1, VOCAB], F32R)
            nc.sync.dma_start(wsum_sb[:], wsum[:])
            bconst_sb = consts.tile([1, VOCAB], F32R)
            nc.sync.dma_start(bconst_sb[:], bconst[:])

            with (
                tc.tile_pool(name="ps_sc", bufs=2, space="PSUM") as ps_sc,
                tc.tile_pool(name="ps_av", bufs=1, space="PSUM") as ps_avp,
                tc.tile_pool(name="ps_den", bufs=1, space="PSUM") as ps_denp,
            ):
                for layer in range(NUM_LAYERS):
                    last = layer == NUM_LAYERS - 1

                    # ---- projections ----
                    qt_sb = qkv.tile([128, S], F32R, tag="q")
                    kt_sb = qkv.tile([128, S], F32R, tag="k")
                    v_sb = qkv.tile([128, NKB, GD], BF16, tag="v")
                    for st in range(NQT):
                        sl = slice(st * QT_W, (st + 1) * QT_W)
                        pq = ps_pr.tile([128, QT_W], F32, tag="proj")
                        for ks in range(2):
                            nc.tensor.matmul(pq[:], wq_sb[:, layer, ks, :],
                                             h_sb[:, ks, sl], start=(ks == 0),
                                             stop=(ks == 1))
                        nc.vector.tensor_copy(qt_sb[:, sl], pq[:])
                        pk = ps_pr.tile([128, QT_W], F32, tag="proj")
                        for ks in range(2):
                            nc.tensor.matmul(pk[:], wk_sb[:, layer, ks, :],
                                             h_sb[:, ks, sl], start=(ks == 0),
                                             stop=(ks == 1))
                        nc.vector.tensor_copy(kt_sb[:, sl], pk[:])
                        pv = ps_pr.tile([128, QT_W], F32, tag="proj")
                        for c in range(4):
                            blk = st * 4 + c
                            csl = slice(c * 128, (c + 1) * 128)
                            for ks in range(2):
                                nc.tensor.matmul(
                                    pv[:, csl],
                                    h_sb[:, ks, blk * 128:(blk + 1) * 128],
                                    wv_sb[:, layer, ks, :],
                                    start=(ks == 0), stop=(ks == 1))
                        nc.vector.tensor_copy(
                            v_sb[:, st * 4:(st + 1) * 4, :], pv[:])

                    # ---- attention + O-projection, per q-tile ----
                    if last:
                        rs_in = [dram.tile([2, 2, 128, QT_W], F32, tag="bnc_rs",
                                           name=f"rsin_{i}") for i in range(2)]
                        rs_out = [dram.tile([2, 128, QT_W], F32, tag="bnc_rso",
                                            name=f"rsout_{i}") for i in range(2)]
                        h4_sb = hpool.tile([128, 2, S_HALF], F32R, tag="h4")
                    else:
                        bounces = [
                            (dram.tile([2, 128, QT_W], F32, tag="bnc",
                                       name=f"bnc_{layer}_{i}"),
                             dram.tile([2, 128, QT_W], F32, tag="bnc_o",
                                       name=f"bnco_{layer}_{i}"))
                            for i in range(NQT)]
                        h_next = hpool.tile([128, 2, S], F32R, tag="h")

                    for qt in range(NQT):
                        qsl = slice(qt * QT_W, (qt + 1) * QT_W)
                        pav = ps_avp.tile([128, QT_W], F32, tag="av")
                        pden = ps_denp.tile([128, QT_W], F32, tag="den")
                        nkb = 4 * qt + 4
                        for kb in range(nkb):
                            ksl = slice(kb * KB, (kb + 1) * KB)
                            j = kb - 4 * qt
                            q0 = 128 * j if j > 0 else 0  # valid q start in tile
                            vsl_q = slice(qt * QT_W + q0, (qt + 1) * QT_W)
                            is_diag = j >= 0
                            # 2 heads per scores tile (2 banks) so the pool
                            # double-buffers in 4 banks; the PE still packs all
                            # 4 heads (disjoint row/col groups, 64-deep queue)
                            for hp in range(2):
                                psc = ps_sc.tile([128, 2, QT_W], F32, tag="sc")
                                for hh in range(2):
                                    h = 2 * hp + hh
                                    nc.tensor.matmul(
                                        psc[:, hh, q0:],
                                        kt_sb[32 * h:32 * h + 32, ksl],
                                        qt_sb[32 * h:32 * h + 32, vsl_q],
                                        start=True, stop=not is_diag,
                                        tile_position=(32 * h, 0),
                                        skip_group_check=True)
                                if is_diag:
                                    for hh in range(2):
                                        nc.tensor.matmul(
                                            psc[:, hh, q0:q0 + 128], ident_sb[:],
                                            masks_sb[:],
                                            start=False, stop=True,
                                            skip_group_check=True)
                                p_sb = ppool.tile([128, 2, QT_W], BF16, tag="p")
                                nc.scalar.activation(p_sb[:, :, q0:],
                                                     psc[:, :, q0:],
                                                     Exp, scale=SCALE)
                                for hh in range(2):
                                    h = 2 * hp + hh
                                    nc.tensor.matmul(
                                        pav[32 * h:32 * h + 32, q0:],
                                        v_sb[:, kb, 32 * h:32 * h + 32],
                                        p_sb[:, hh, q0:],
                                        start=(kb == 0), stop=(kb == nkb - 1),
                                        tile_position=(0, 32 * h),
                                        skip_group_check=True)
                                    nc.tensor.matmul(
                                        pden[32 * h:32 * h + 32, q0:],
                                        ones32_sb[:], p_sb[:, hh, q0:],
                                        start=(kb == 0), stop=(kb == nkb - 1),
                                        tile_position=(0, 32 * h),
                                        skip_group_check=True)
                        recip_sb = attp.tile([128, QT_W], F32R, tag="recip")
                        with nc.allow_low_precision(reason="softmax denom recip"):
                            nc.vector.reciprocal(recip_sb[:], pden[:])
                        att_sb = attp.tile([128, QT_W], F32R, tag="att")
                        nc.vector.tensor_tensor(att_sb[:], pav[:], recip_sb[:], MUL)

                        # O-projection partial: h_part[o, q] for o in 2 blocks
                        for mb in range(2):
                            po = ps_pr.tile([128, QT_W], F32, tag="proj")
                            nc.tensor.matmul(po[:], wo_sb[:, layer, mb, :],
                                             att_sb[:], start=True, stop=True)
                            o_sb = attp.tile([128, QT_W], F32, tag="osb")
                            nc.vector.tensor_copy(o_sb[:], po[:])
                            if last:
                                # interleaved quarters: RS_A=[qt0|qt1] after
                                # qt1, RS_B=[qt3|qt2] after qt3 -> every core
                                # gets half its head work early (tail balance)
                                ri, slot = {0: (0, 0), 1: (0, 1),
                                            2: (1, 1), 3: (1, 0)}[qt]
                                nc.sync.dma_start(
                                    rs_in[ri][slot, mb, :, :], o_sb[:])
                            else:
                                nc.sync.dma_start(bounces[qt][0][mb, :, :], o_sb[:])
                        if not last:
                            bi, bo = bounces[qt]
                            nc.gpsimd.collective_compute(
                                "AllReduce", mybir.AluOpType.add,
                                replica_groups=groups,
                                ins=[bi.opt()], outs=[bo.opt()])
                            for mb in range(2):
                                nc.sync.dma_start(
                                    h_next[:, mb, qsl],
                                    bo[mb, :, :].bitcast(F32R))
                        elif qt % 2 == 1:
                            i = qt // 2
                            nc.gpsimd.collective_compute(
                                "ReduceScatter", mybir.AluOpType.add,
                                replica_groups=groups,
                                ins=[rs_in[i].opt()], outs=[rs_out[i].opt()])
                            csl = slice(i * QT_W, (i + 1) * QT_W)
                            for mb in range(2):
                                nc.sync.dma_start(
                                    h4_sb[:, mb, csl],
                                    rs_out[i][mb, :, :].bitcast(F32R))

                    if not last:
                        h_sb = h_next

            # ---- final LayerNorm (stats) + vocab head ----
            psf_cm = tc.tile_pool(name="ps_fin", bufs=2, space="PSUM")
            psf = psf_cm.__enter__()
            sq_sb = hpool.tile([128, 2, S_HALF], F32R, tag="h4sq")
            psf2_cm = tc.tile_pool(name="ps_fin2", bufs=2, space="PSUM")
            psf2 = psf2_cm.__enter__()
            negm_sb = small.tile([1, S_HALF], F32R, tag="negm")
            m2_sb = small.tile([1, S_HALF], F32, tag="m2")
            var_sb = small.tile([1, S_HALF], F32, tag="var")
            std_sb = small.tile([1, S_HALF], F32R, tag="std")
            rstd_sb = small.tile([1, S_HALF], F32, tag="rstd")
            rstd_dram = dram.tile([1, S_HALF], F32, tag="rstd_d")
            rstd_col = small.tile([128, S_HALF // 128], F32, tag="rstd_c")
            # per 512-wide half: stats -> var -> rstd -> transpose round-trip,
            # so half 0 head matmuls can start while half 1 stats still run
            for sh in range(S_HALF // 512):
                shsl = slice(sh * 512, (sh + 1) * 512)
                nc.vector.tensor_tensor(sq_sb[:, :, shsl], h4_sb[:, :, shsl],
                                        h4_sb[:, :, shsl], MUL)
                ps_m = psf.tile([1, 512], F32, tag="stat", name=f"ps_m{sh}")
                ps_s = psf.tile([1, 512], F32, tag="stat", name=f"ps_s{sh}")
                for ks in range(2):
                    nc.tensor.matmul(ps_m[:], ones1_sb[:],
                                     h4_sb[:, ks, shsl],
                                     start=(ks == 0), stop=(ks == 1),
                                     skip_group_check=True)
                    nc.tensor.matmul(ps_s[:], ones1_sb[:],
                                     sq_sb[:, ks, shsl],
                                     start=(ks == 0), stop=(ks == 1),
                                     skip_group_check=True)
                # var = sumsq/D - (sum/D)^2; std = sqrt(var+eps); rstd = 1/std
                nc.vector.tensor_scalar(negm_sb[:, shsl], ps_m[:],
                                        -1.0 / D_MODEL, None, MUL)
                nc.vector.tensor_tensor(m2_sb[:, shsl], negm_sb[:, shsl],
                                        negm_sb[:, shsl], MUL)
                nc.vector.tensor_scalar(var_sb[:, shsl], ps_s[:],
                                        1.0 / D_MODEL, None, MUL)
                nc.vector.tensor_tensor(var_sb[:, shsl], var_sb[:, shsl],
                                        m2_sb[:, shsl], SUB)
                nc.vector.tensor_scalar(var_sb[:, shsl], var_sb[:, shsl],
                                        LN_EPS, None, ADD)
                nc.scalar.activation(std_sb[:, shsl], var_sb[:, shsl], Sqrt)
                nc.vector.reciprocal(rstd_sb[:, shsl], std_sb[:, shsl])
                nc.sync.dma_start(rstd_dram[:, shsl], rstd_sb[:, shsl])
                nc.sync.dma_start(
                    rstd_col[:, sh * 4:(sh + 1) * 4],
                    rstd_dram[0, shsl].rearrange("(b p) -> p b", p=128))

            vh = VOCAB // 2  # 500
            for sb in range(S_HALF // 128):
                ssl = slice(sb * 128, (sb + 1) * 128)
                ph = psf2.tile([128, 2, 512], F32, tag="head")
                for v0 in range(2):
                    vsl = slice(v0 * vh, (v0 + 1) * vh)
                    for ks in range(2):
                        nc.tensor.matmul(ph[:, v0, :vh], h4_sb[:, ks, ssl],
                                         whd_sb[:, ks, vsl],
                                         start=(ks == 0), stop=False,
                                         skip_group_check=True)
                    nc.tensor.matmul(ph[:, v0, :vh], negm_sb[:, ssl],
                                     wsum_sb[:, vsl], start=False, stop=False,
                                     skip_group_check=True)
                    nc.tensor.matmul(ph[:, v0, :vh], std_sb[:, ssl],
                                     bconst_sb[:, vsl], start=False, stop=True,
                                     skip_group_check=True)
                out_sb = logitp.tile([128, VOCAB], F32, tag="logit")
                for v0 in range(2):
                    vsl = slice(v0 * vh, (v0 + 1) * vh)
                    nc.vector.tensor_scalar(out_sb[:, vsl], ph[:, v0, :vh],
                                            rstd_col[:, sb:sb + 1], None, MUL)
                nc.sync.dma_start(y[ssl, :], out_sb[:])
            psf2_cm.__exit__(None, None, None)
            psf_cm.__exit__(None, None, None)

    nc.compile()
    return nc


def _host_prep(x, embed, Wq, Wk, Wv, Wo, gamma, beta, Whead, bhead):
    """Build the 8 per-core input maps (numpy, all host-side)."""
    f32 = np.float32
    bf16 = np.float16

    def to_lhsT(w):  # [o, i] -> [i_in 128, i_out 2, o]
        return np.ascontiguousarray(
            w.T.reshape(2, 128, w.shape[0]).transpose(1, 0, 2)).astype(f32)

    p = np.arange(128)[:, None]
    f = np.arange(128)[None, :]
    masks = np.where(p > f, MASK_NEG, 0.0).astype(bf16)
    identb = np.eye(128, dtype=bf16)
    ones32 = np.ones((128, 32), bf16)
    ones1r = np.ones((128, 1), f32)

    gamma = np.asarray(gamma, f32)
    beta = np.asarray(beta, f32)
    Whead_g = np.asarray(Whead, f32) * gamma[None, :]
    whd = np.ascontiguousarray(
        Whead_g.T.reshape(2, 128, VOCAB).transpose(1, 0, 2)).astype(f32)
    wsum = Whead_g.sum(axis=1).reshape(1, VOCAB).astype(f32)
    bconst = (np.asarray(Whead, f32).T * beta[:, None]).sum(axis=0)
    bconst = (bconst + np.asarray(bhead, f32)).reshape(1, VOCAB).astype(f32)

    embed = np.asarray(embed, f32)
    x = np.asarray(x)
    in_maps = []
    for c in range(N_CORES):
        b, g = c // 2, c % 2
        gsl = slice(128 * g, 128 * (g + 1))
        h0 = embed[x[b]]  # [S, 256]
        h0t = np.ascontiguousarray(
            h0.T.reshape(2, 128, S).transpose(1, 0, 2)).astype(f32)
        wq_c = np.stack([to_lhsT(np.asarray(Wq[l], f32)[gsl, :])
                         for l in range(NUM_LAYERS)])
        wk_c = np.stack([to_lhsT(np.asarray(Wk[l], f32)[gsl, :])
                         for l in range(NUM_LAYERS)])
        wv_c = np.stack([to_lhsT(np.asarray(Wv[l], f32)[gsl, :])
                         for l in range(NUM_LAYERS)])
        # Wo: columns gsl -> lhsT [i_own 128, o_out 2, o_in 128]
        wo_c = np.stack([
            np.ascontiguousarray(
                np.asarray(Wo[l], f32)[:, gsl].T.reshape(128, 2, 128))
            for l in range(NUM_LAYERS)])
        in_maps.append({
            "h0t": h0t, "wq": wq_c, "wk": wk_c, "wv": wv_c, "wo": wo_c,
            "whd": whd, "wsum": wsum, "bconst": bconst,
            "masks": np.asarray(masks), "identb": identb, "ones32": ones32,
            "ones1r": ones1r,
        })
    return in_maps


def kernel(x, embed, Wq, Wk, Wv, Wo, gamma, beta, Whead, bhead):
    from concourse.bass_utils import run_bass_kernel_spmd

    if "nc" not in _prog_cache:
        _prog_cache["nc"] = _build_program()
    nc = _prog_cache["nc"]

    in_maps = _host_prep(x, embed, Wq, Wk, Wv, Wo, gamma, beta, Whead, bhead)
    res = run_bass_kernel_spmd(nc, in_maps, core_ids=list(range(N_CORES)))

    out = np.empty((B, S, VOCAB), np.float32)
    Q = QT_W  # 512
    for c in range(N_CORES):
        b, r = c // 2, c % 2
        yc = res.results[c]["y"]
        if r == 0:   # rank 0 owns quarters 0 and 3
            out[b, 0:Q, :] = yc[0:Q]
            out[b, 3 * Q:4 * Q, :] = yc[Q:2 * Q]
        else:        # rank 1 owns quarters 1 and 2
            out[b, Q:2 * Q, :] = yc[0:Q]
            out[b, 2 * Q:3 * Q, :] = yc[Q:2 * Q]
    return out


if __name__ == "__main__":
    rng = np.random.default_rng(0)
    inputs = {
        "x": rng.integers(0, VOCAB, (B, S)).astype(np.int32),
        "embed": rng.standard_normal((VOCAB, D_MODEL), np.float32),
        "Wq": (rng.standard_normal((NUM_LAYERS, D_MODEL, D_MODEL), np.float32) / 16),
        "Wk": (rng.standard_normal((NUM_LAYERS, D_MODEL, D_MODEL), np.float32) / 16),
        "Wv": (rng.standard_normal((NUM_LAYERS, D_MODEL, D_MODEL), np.float32) / 16),
        "Wo": (rng.standard_normal((NUM_LAYERS, D_MODEL, D_MODEL), np.float32) / 16),
        "gamma": np.ones(D_MODEL, np.float32),
        "beta": np.zeros(D_MODEL, np.float32),
        "Whead": (rng.standard_normal((VOCAB, D_MODEL), np.float32) / 16),
        "bhead": np.zeros(VOCAB, np.float32),
    }
    y = kernel(**inputs)
    print("out", y.shape, y.dtype, float(np.abs(y).max()))

